# revision 12
# baseline (speedup 1.0000x reference)
"""Trainium2 Bass kernel for nn_ExtractionLayer (v3, windowed sparsity).

metric[b,v,f] = sum_p amp[b,f,p] * exp(-c*(vol[v]*filt[f] - q[b,p])^2)
  amp = softmax_p(logits[b,f,p]),  c = 0.5/(sigma+0.001)^2

Sharding: data-parallel over batch B=32 -> 4 b's per core on 8 cores.

v2 recap (transposed layout): chunks (f, vh) put 128 v's on PSUM
partitions and (b,p) on the free axis; a K=12 bf16 matmul per chunk
computes S = x^2 - 2qx + q^2 - lnamp/c (softmax amp folded into the
exponent), ACT does E = exp(-c*S), DVE does the segmented p-sum.

v3 adds windowed sparsity: exp(-c*d^2) < 1e-6 once |d| > sqrt(14/c)
(~0.15 here), so for each chunk only the q[b,p] inside the chunk's
x-range (+/- thr) can contribute. v is pre-sorted by vol so each
v-half spans ~0.5*filt[f] in x. Per chunk we keep
Ks = ceil8(max_b #selected) p-slots per b (max over the GLOBAL batch
so all 8 SPMD cores share one schedule; dropped terms < 64*e^-14).
Mean Ks ~31 of 64 => ~2x fewer exp/matmul/reduce columns.

Scheduling: chunks sorted by Ks, paired; pair = (band0 chunk, band1
chunk) with equal padded Ks. Groups of pairs fill a [128, 2048] PSUM
tile: band 0 -> cols [0,1024) (banks 0-1), band 1 -> [1024,2048)
(banks 2-3) -- concurrent row-tile matmuls must never share a PSUM
bank. All chunks of a group share one Ks (padded to the group max) so
the group needs one EXP and one fp16 halving add + one 16-ish-wide
reduce. Pad columns carry phi=100 so exp() -> 0 exactly.

ALL small tensors are precomputed on host in fp64 and shipped as two
bf16 input tiles; the schedule is baked per (sigma, selection counts)
and cached.
"""

import sys

for _p in ("/opt/trn_rl_repo", "/root/.axon_site/_ro/trn_rl_repo"):
    if _p not in sys.path:
        sys.path.append(_p)

import numpy as np
import ml_dtypes

BF16 = ml_dtypes.bfloat16

B, V, F, P = 32, 256, 128, 64
NCORES = 8
B_LOC = B // NCORES          # 4 batches per core
NCH = 2 * F                  # 256 chunks: (f, vh)
NK = 12                      # matmul contraction rows
HALF = 1024                  # psum cols per band-half (2 banks)
THR_LN = 14.0                # keep q with c*(x-q)^2 <= THR_LN at window edge
PAD_PHI = 100.0              # phi for padding columns -> exp(-c*100) == 0

_cache: dict = {}


class Schedule:
    """Data-dependent but core-independent processing plan."""

    def __init__(self, Ks_chunk, order):
        # order: list of chunk ids sorted by Ks desc; pairs = (order[2i],
        # order[2i+1]) with pair Ks = max of the two; groups pack pairs
        # with a shared (max) Ks such that npairs*4*Ks <= HALF.
        self.pairs = []                  # (chunkA, chunkB, Ks_pair)
        for i in range(0, NCH, 2):
            a, b = order[i], order[i + 1]
            self.pairs.append((a, b, max(Ks_chunk[a], Ks_chunk[b])))
        self.groups = []                 # list of (pair_lo, npairs, Ks_grp)
        i = 0
        while i < len(self.pairs):
            Kg = self.pairs[i][2]
            n = 1
            while (i + n < len(self.pairs)
                   and (n + 1) * 4 * Kg <= HALF):
                n += 1
            self.groups.append((i, n, Kg))
            i += n
        # flat emission order of chunks: per group, per pair: A then B
        self.chunks = []                 # (chunk_id, band, group, slot)
        for gi, (plo, npair, Kg) in enumerate(self.groups):
            for s in range(npair):
                a, b, _ = self.pairs[plo + s]
                self.chunks.append((a, 0, gi, s))
                self.chunks.append((b, 1, gi, s))
        # R column map: chunk -> base col of its 4 b sums.
        # reduce out for group gi half H: cols rbase(g) + H*4*npair + s*4 + b
        self.rbase = []
        acc = 0
        for (plo, npair, Kg) in self.groups:
            self.rbase.append(acc)
            acc += 8 * npair
        self.rtot = acc                  # == 8 * 128 == 1024
        self.rcol = {}                   # chunk_id -> base col
        for (cid, band, gi, s) in self.chunks:
            plo, npair, Kg = self.groups[gi]
            self.rcol[cid] = self.rbase[gi] + band * 4 * npair + s * 4
        # wmv col offset per chunk (emission order, 4*Ks_grp cols each)
        self.woff = {}
        acc = 0
        for (cid, band, gi, s) in self.chunks:
            Kg = self.groups[gi][2]
            self.woff[cid] = acc
            acc += 4 * Kg
        self.wtot = acc
        # xst col offset: 128 per chunk in emission order
        self.xoff = {cid: k * 128 for k, (cid, _, _, _) in
                     enumerate(self.chunks)}
        self.key = (tuple(Ks_chunk), tuple(order))


def _build(minus_c, sched):
    import concourse.tile as tile
    from concourse import bacc, mybir

    fp32 = mybir.dt.float32
    fp16 = mybir.dt.float16
    bf16 = mybir.dt.bfloat16
    AF = mybir.ActivationFunctionType
    OP = mybir.AluOpType
    import concourse.bass as bass

    nc = bacc.Bacc("TRN2", target_bir_lowering=False, debug=False,
                   num_devices=NCORES)

    d_xst = nc.dram_tensor("xst", [44, NCH * 128], bf16,
                           kind="ExternalInput")
    d_wmv = nc.dram_tensor("wmv", [44, sched.wtot], bf16,
                           kind="ExternalInput")
    d_out = nc.dram_tensor("out", [128, sched.rtot], fp32,
                           kind="ExternalOutput")

    ngroups = len(sched.groups)
    # chunk list per group for emission
    by_group = [[] for _ in range(ngroups)]
    for (cid, band, gi, s) in sched.chunks:
        by_group[gi].append((cid, band, s))

    with tile.TileContext(nc) as tc:
        with (
            tc.tile_pool(name="const", bufs=1) as cp,
            tc.tile_pool(name="ering", bufs=2) as ep,
            tc.tile_pool(name="e2ring", bufs=2) as ep2,
            tc.tile_pool(name="psS", bufs=2, space=bass.MemorySpace.PSUM) as psS,
        ):
            warm = cp.tile([1, 2], fp32, tag="warm")

            xst = cp.tile([44, NCH * 128], bf16, tag="xst")
            wmv = cp.tile([44, sched.wtot], bf16, tag="wmv")
            R = cp.tile([128, sched.rtot], fp32, tag="R")

            def grange(ga, gb):
                """xst/wmv col ranges covering groups [ga, gb)."""
                ca = by_group[ga][0][0]
                cz = by_group[gb - 1][-1][0]
                Kg = sched.groups[gb - 1][2]
                return (sched.xoff[ca], sched.xoff[cz] + 128,
                        sched.woff[ca], sched.woff[cz] + 4 * Kg)

            # late ~40% of both tensors: two big DMAs on the ACT queue,
            # issued first so the transfer runs in the background
            gsplit = max(1, min(ngroups - 1, int(ngroups * 0.6)))
            x0, x1, w0, w1 = grange(gsplit, ngroups)
            nc.scalar.dma_start(xst[:, x0:x1], d_xst.ap()[:, x0:x1])
            nc.scalar.dma_start(wmv[:, w0:w1], d_wmv.ap()[:, w0:w1])

            nc.vector.memset(warm[:, :], 0.0)
            nc.scalar.activation(warm[:, 0:1], warm[:, 1:2], AF.Exp)

            # early groups stream in fine pieces on sync (xst) + gpsimd (wmv)
            gsz = [1, 1, 1, 1, 2, 2]
            while sum(gsz) < gsplit:
                gsz.append(min(3, gsplit - sum(gsz)))
            g0 = 0
            for ng in gsz:
                gb = min(g0 + ng, gsplit)
                x0, x1, w0, w1 = grange(g0, gb)
                nc.sync.dma_start(xst[:, x0:x1], d_xst.ap()[:, x0:x1])
                nc.gpsimd.dma_start(wmv[:, w0:w1], d_wmv.ap()[:, w0:w1])
                g0 = gb

            ocursor = 0
            for gi in range(ngroups):
                plo, npair, Kg = sched.groups[gi]
                h = npair * 4 * Kg       # cols per half
                sS = psS.tile([128, 2 * HALF], fp32, tag="S", name="sS")
                for (cid, band, s) in by_group[gi]:
                    r0 = 32 * band
                    xo = sched.xoff[cid]
                    wo = sched.woff[cid]
                    pc = band * HALF + s * 4 * Kg
                    nc.tensor.matmul(
                        sS[:, pc:pc + 4 * Kg],
                        xst[r0:r0 + NK, xo:xo + 128],
                        wmv[r0:r0 + NK, wo:wo + 4 * Kg],
                        start=True, stop=True,
                        tile_position=(r0, 0),
                    )
                E = ep.tile([128, 2 * HALF], fp16, tag="E", name="E")
                Sv = sS[:, :].rearrange("p (u x) -> p u x", u=2)[:, :, 0:h]
                Ev = E[:, :].rearrange("p (u x) -> p u x", u=2)[:, :, 0:h]
                nc.scalar.activation(Ev, Sv, AF.Exp, scale=float(minus_c))
                # p-sum: one fp16 halving add (2x mode) + one Kg/2 reduce
                nseg = npair * 4
                E4 = (E[:, :].rearrange("p (u y) -> p u y", u=2)
                      [:, :, 0:nseg * Kg]
                      .rearrange("p u (s x) -> p u s x", x=Kg))
                E2 = ep2.tile([128, HALF], fp16, tag="E2", name="E2")
                E2v = (E2[:, :].rearrange("p (u y) -> p u y", u=2)
                       [:, :, 0:nseg * (Kg // 2)]
                       .rearrange("p u (s x) -> p u s x", x=Kg // 2))
                nc.vector.tensor_tensor(E2v, E4[:, :, :, 0:Kg // 2],
                                        E4[:, :, :, Kg // 2:Kg], OP.add)
                nc.vector.tensor_reduce(
                    R[:, sched.rbase[gi]:sched.rbase[gi] + 8 * npair]
                    .rearrange("p (u s) -> p u s", u=2),
                    E2v, mybir.AxisListType.X, OP.add)
                # stream out finished R spans every few groups
                rend = sched.rbase[gi] + 8 * npair
                if gi % 3 == 2 or gi == ngroups - 1:
                    nc.sync.dma_start(d_out.ap()[:, ocursor:rend],
                                      R[:, ocursor:rend])
                    ocursor = rend

    nc.compile()
    return nc


def _get_nc(minus_c, sched):
    key = (float(minus_c), sched.key)
    if key not in _cache:
        _cache[key] = _build(minus_c, sched)
    return _cache[key]


def _split3(v):
    """3-way bf16 split of an fp64 array: h + m + l ~= v to ~24 bits."""
    h = v.astype(BF16)
    r = v - h.astype(np.float64)
    m = r.astype(BF16)
    r2 = r - m.astype(np.float64)
    l = r2.astype(BF16)
    return h, m, l


def kernel(q2_obs_scaled, amplitude_logits, volumes, filters, sigma,
           _trace=False, _tmpdir=None):
    from concourse.bass_utils import run_bass_kernel_spmd

    sig = float(np.asarray(sigma).reshape(()))
    minus_c = -0.5 / (sig + 0.001) ** 2
    c = -minus_c
    thr = np.sqrt(THR_LN / c)

    q = np.asarray(q2_obs_scaled, np.float64)                    # (B, P)
    lg = np.asarray(amplitude_logits, np.float64).reshape(B, F, P)
    vol = np.asarray(volumes, np.float64).reshape(V)
    fil = np.asarray(filters, np.float64).reshape(F)

    mx = lg.max(axis=2, keepdims=True)
    lnamp = lg - (mx + np.log(np.exp(lg - mx).sum(axis=2, keepdims=True)))

    # ---- schedule: windowed selection, global over the batch ----
    vperm = np.argsort(vol, kind="stable")
    vs = vol[vperm]
    xs = vs[:, None] * fil[None, :]                              # (V, F)
    sel = [None] * NCH                                           # (B, P) bool
    Ks_chunk = [0] * NCH
    for cid in range(NCH):
        f, vh = cid >> 1, cid & 1
        xw = xs[vh * 128:(vh + 1) * 128, f]
        lo, hi = xw.min() - thr, xw.max() + thr
        m = (q >= lo) & (q <= hi)                                # (B, P)
        sel[cid] = m
        # power-of-two K so same-size PSUM slots never straddle a bank
        n = int(m.sum(axis=1).max())
        Ks_chunk[cid] = next(k for k in (8, 16, 32, 64) if k >= n)
    order = sorted(range(NCH), key=lambda cix: -Ks_chunk[cix])
    sched = Schedule(Ks_chunk, order)
    nc = _get_nc(minus_c, sched)

    # ---- stationary x-side tile (shared by all cores) ----
    xst = np.zeros((44, NCH * 128), dtype=BF16)
    for (cid, band, gi, s) in sched.chunks:
        f, vh = cid >> 1, cid & 1
        xw = xs[vh * 128:(vh + 1) * 128, f]                      # (128,)
        x2h, x2m, x2l = _split3(xw * xw)
        xh, xm, xl = _split3(xw)
        ones = np.ones(128, dtype=BF16)
        rows = [x2h, x2m, x2l, xh, xh, xh, xm, xm, xl, ones, ones, ones]
        xo = sched.xoff[cid]
        for r, arr in enumerate(rows):
            xst[32 * band + r, xo:xo + 128] = arr

    # ---- per-core moving q-side tiles ----
    wh_a, wm_a, wl_a = _split3(-2.0 * q)                         # (B, P)
    phi = q[:, None, :] ** 2 - lnamp / c                         # (B, F, P)
    ph_a, pm_a, pl_a = _split3(phi)

    in_maps = []
    for i in range(NCORES):
        wmv = np.zeros((44, sched.wtot), dtype=BF16)
        for (cid, band, gi, s) in sched.chunks:
            f = cid >> 1
            Kg = sched.groups[gi][2]
            wo = sched.woff[cid]
            r0 = 32 * band
            for bl in range(B_LOC):
                bg = B_LOC * i + bl
                ps = np.nonzero(sel[cid][bg])[0]
                n = len(ps)
                col = wo + bl * Kg
                wmv[r0 + 0, col:col + Kg] = 1.0
                wmv[r0 + 1, col:col + Kg] = 1.0
                wmv[r0 + 2, col:col + Kg] = 1.0
                wmv[r0 + 3, col:col + n] = wh_a[bg, ps]
                wmv[r0 + 4, col:col + n] = wm_a[bg, ps]
                wmv[r0 + 5, col:col + n] = wl_a[bg, ps]
                wmv[r0 + 6, col:col + n] = wh_a[bg, ps]
                wmv[r0 + 7, col:col + n] = wm_a[bg, ps]
                wmv[r0 + 8, col:col + n] = wh_a[bg, ps]
                wmv[r0 + 9, col:col + n] = ph_a[bg, f, ps]
                wmv[r0 + 10, col:col + n] = pm_a[bg, f, ps]
                wmv[r0 + 11, col:col + n] = pl_a[bg, f, ps]
                if n < Kg:
                    wmv[r0 + 9, col + n:col + Kg] = PAD_PHI
        in_maps.append({"xst": xst, "wmv": wmv})

    kw = {}
    if _trace:
        kw = {"trace": True, "tmpdir": _tmpdir}
    res = run_bass_kernel_spmd(nc, in_maps, core_ids=list(range(NCORES)), **kw)

    # ---- host unpack: R[v'(sorted), rcol[cid]+b] -> out[b, v, f] ----
    vback = vperm.reshape(2, 128)                                # vh, v'
    out = np.empty((B, V, F), dtype=np.float32)
    rc = np.array([sched.rcol[cid] for cid in range(NCH)])       # (NCH,)
    for i in range(NCORES):
        R = res.results[i]["out"]                                # (128, rtot)
        cols = rc[:, None] + np.arange(B_LOC)[None, :]           # (NCH, 4)
        Rg = R[:, cols]                                          # (128,NCH,4)
        for bl in range(B_LOC):
            o = out[B_LOC * i + bl]                              # (V, F)
            g = Rg[:, :, bl].reshape(128, F, 2)                  # v', f, vh
            for vh in range(2):
                o[vback[vh], :] = g[:, :, vh]
    if _trace:
        return out, res
    return out


# revision 13
# speedup vs baseline: 1.5791x; 1.5791x over previous
"""Trainium2 Bass kernel for nn_ExtractionLayer (v4, 4-band windowed sparsity).

metric[b,v,f] = sum_p amp[b,f,p] * exp(-c*(vol[v]*filt[f] - q[b,p])^2)
  amp = softmax_p(logits[b,f,p]),  c = 0.5/(sigma+0.001)^2

Sharding: data-parallel over batch B=32 -> 4 b's per core on 8 cores.

Transposed layout: chunks (f, vh) put 128 v's (vol-sorted) on PSUM
partitions and selected (b,p) columns on the free axis. A K=12 bf16
matmul per chunk computes S = x^2 - 2qx + q^2 - lnamp/c (softmax amp
folded into the exponent via ln), ACT does E = exp(-c*S) in ~2048-col
group instructions, DVE does the segmented p-sum (one fp16 halving add
in 2x mode + one short reduce per group).

Windowed sparsity: exp(-c*d^2) < 1e-6 once |d| > sqrt(14/c), so each
chunk keeps only Ks = pow2ceil(max_b #{p: q[b,p] in x-window}) p-slots
per b (max over the GLOBAL batch so all 8 SPMD cores share one
schedule; pow2 Ks so equal PSUM slots never straddle a bank).

4 PE bands: chunks rotate tile_position row 0/32/64/96; band i writes
PSUM bank i of the group tile (concurrent row-tile matmuls must never
share a PSUM bank). The stationary/moving tiles hold band data at
partition blocks 0/32/64/96, which cuts per-partition DMA bytes ~2.5x
vs 2 bands -- input DMA then easily outruns the ~1.9us/group pipeline.

ALL small tensors are precomputed on host in fp64; the schedule is
baked per (sigma, selection counts) and cached.
"""

import sys

for _p in ("/opt/trn_rl_repo", "/root/.axon_site/_ro/trn_rl_repo"):
    if _p not in sys.path:
        sys.path.append(_p)

import numpy as np
import ml_dtypes

BF16 = ml_dtypes.bfloat16

B, V, F, P = 32, 256, 128, 64
NCORES = 8
B_LOC = B // NCORES          # 4 batches per core
NCH = 2 * F                  # 256 chunks: (f, vh)
NK = 12                      # matmul contraction rows
BANK = 512                   # psum cols per bank == per band-quarter
THR_LN = 14.0                # keep q with c*(x-q)^2 <= THR_LN at window edge
PAD_PHI = 100.0              # phi for padding columns -> exp(-c*100) == 0

_cache: dict = {}


class Schedule:
    """Data-dependent but core-independent processing plan.

    Chunks sorted by Ks desc, packed into groups of 4*spb slots
    (spb = 512 // (4*Kg) slots per bank, 4 banks). cid == -1 marks a
    dummy pad chunk (all-pad columns, output discarded).
    """

    def __init__(self, Ks_chunk, order):
        self.groups = []        # (Kg, spb, [cids (len 4*spb, -1 pads)])
        i = 0
        while i < NCH:
            Kg = Ks_chunk[order[i]]
            spb = BANK // (4 * Kg)
            cap = 4 * spb
            cids = order[i:i + cap]
            i += len(cids)
            cids = list(cids) + [-1] * (cap - len(cids))
            self.groups.append((Kg, spb, cids))
        # per-chunk placement
        self.place = {}         # cid -> (gi, band, slot)
        self.xoff = {}          # cid -> xst col offset (128 wide)
        self.woff = {}          # cid -> wmv col offset (4*Kg wide)
        self.rcol = {}          # cid -> R col base (4 wide)
        self.rbase = []
        xslot = 0
        wcur = 0
        racc = 0
        self.xslots = []        # per group: (xoff0, woff0) for DMA ranges
        for gi, (Kg, spb, cids) in enumerate(self.groups):
            self.rbase.append(racc)
            self.xslots.append((xslot * 128, wcur))
            for idx, cid in enumerate(cids):
                band, slot = idx % 4, idx // 4
                key = cid if cid >= 0 else ("pad", gi, idx)
                self.place[key] = (gi, band, slot)
                self.xoff[key] = (xslot + slot) * 128
                self.woff[key] = wcur + slot * 4 * Kg
                self.rcol[key] = racc + band * 4 * spb + slot * 4
            xslot += spb
            wcur += spb * 4 * Kg
            racc += 16 * spb
        self.xtot = xslot * 128
        self.wtot = wcur
        self.rtot = racc
        self.key = (tuple(Ks_chunk), tuple(order))


def _build(minus_c, sched):
    import concourse.tile as tile
    from concourse import bacc, mybir

    fp32 = mybir.dt.float32
    fp16 = mybir.dt.float16
    bf16 = mybir.dt.bfloat16
    AF = mybir.ActivationFunctionType
    OP = mybir.AluOpType
    import concourse.bass as bass

    nc = bacc.Bacc("TRN2", target_bir_lowering=False, debug=False,
                   num_devices=NCORES)

    d_xst = nc.dram_tensor("xst", [108, sched.xtot], bf16,
                           kind="ExternalInput")
    d_wmv = nc.dram_tensor("wmv", [108, sched.wtot], bf16,
                           kind="ExternalInput")
    d_out = nc.dram_tensor("out", [128, sched.rtot], fp32,
                           kind="ExternalOutput")

    ngroups = len(sched.groups)

    with tile.TileContext(nc) as tc:
        with (
            tc.tile_pool(name="const", bufs=1) as cp,
            tc.tile_pool(name="ering", bufs=2) as ep,
            tc.tile_pool(name="e2ring", bufs=2) as ep2,
            tc.tile_pool(name="psS", bufs=2, space=bass.MemorySpace.PSUM) as psS,
        ):
            warm = cp.tile([1, 2], fp32, tag="warm")
            nc.vector.memset(warm[:, :], 0.0)
            nc.scalar.activation(warm[:, 0:1], warm[:, 1:2], AF.Exp)

            xst = cp.tile([108, sched.xtot], bf16, tag="xst")
            wmv = cp.tile([108, sched.wtot], bf16, tag="wmv")
            R = cp.tile([128, sched.rtot], fp32, tag="R")

            # input pieces by groups: fine first, then coarse
            gsz = [1, 1, 1, 1, 2, 2]
            while sum(gsz) < ngroups:
                gsz.append(min(3, ngroups - sum(gsz)))
            g0 = 0
            for ng in gsz:
                gb = min(g0 + ng, ngroups)
                x0, w0 = sched.xslots[g0]
                if gb < ngroups:
                    x1, w1 = sched.xslots[gb]
                else:
                    x1, w1 = sched.xtot, sched.wtot
                nc.sync.dma_start(xst[:, x0:x1], d_xst.ap()[:, x0:x1])
                nc.gpsimd.dma_start(wmv[:, w0:w1], d_wmv.ap()[:, w0:w1])
                g0 = gb

            ocursor = 0
            for gi in range(ngroups):
                Kg, spb, cids = sched.groups[gi]
                h = spb * 4 * Kg          # cols per bank (== 512 if full)
                sS = psS.tile([128, 4 * BANK], fp32, tag="S", name="sS")
                for idx, cid in enumerate(cids):
                    key = cid if cid >= 0 else ("pad", gi, idx)
                    band, slot = idx % 4, idx // 4
                    r0 = 32 * band
                    xo = sched.xoff[key]
                    wo = sched.woff[key]
                    pc = band * BANK + slot * 4 * Kg
                    nc.tensor.matmul(
                        sS[:, pc:pc + 4 * Kg],
                        xst[r0:r0 + NK, xo:xo + 128],
                        wmv[r0:r0 + NK, wo:wo + 4 * Kg],
                        start=True, stop=True,
                        tile_position=(r0, 0),
                    )
                E = ep.tile([128, 4 * BANK], fp16, tag="E", name="E")
                Sv = sS[:, :].rearrange("p (u x) -> p u x", u=4)[:, :, 0:h]
                Ev = E[:, :].rearrange("p (u x) -> p u x", u=4)[:, :, 0:h]
                nc.scalar.activation(Ev, Sv, AF.Exp, scale=float(minus_c))
                # p-sum: one fp16 halving add (2x mode) + one Kg/2 reduce
                nseg = spb * 4
                E4 = (E[:, :].rearrange("p (u y) -> p u y", u=4)
                      [:, :, 0:nseg * Kg]
                      .rearrange("p u (s x) -> p u s x", x=Kg))
                E2 = ep2.tile([128, 2 * BANK], fp16, tag="E2", name="E2")
                E2v = (E2[:, :].rearrange("p (u y) -> p u y", u=4)
                       [:, :, 0:nseg * (Kg // 2)]
                       .rearrange("p u (s x) -> p u s x", x=Kg // 2))
                nc.vector.tensor_tensor(E2v, E4[:, :, :, 0:Kg // 2],
                                        E4[:, :, :, Kg // 2:Kg], OP.add)
                nc.vector.tensor_reduce(
                    R[:, sched.rbase[gi]:sched.rbase[gi] + 16 * spb]
                    .rearrange("p (u s) -> p u s", u=4),
                    E2v, mybir.AxisListType.X, OP.add)
                rend = sched.rbase[gi] + 16 * spb
                if gi % 3 == 2 or gi == ngroups - 1:
                    nc.sync.dma_start(d_out.ap()[:, ocursor:rend],
                                      R[:, ocursor:rend])
                    ocursor = rend

    nc.compile()
    return nc


def _get_nc(minus_c, sched):
    key = (float(minus_c), sched.key)
    if key not in _cache:
        _cache[key] = _build(minus_c, sched)
    return _cache[key]


def _split3(v):
    """3-way bf16 split of an fp64 array: h + m + l ~= v to ~24 bits."""
    h = v.astype(BF16)
    r = v - h.astype(np.float64)
    m = r.astype(BF16)
    r2 = r - m.astype(np.float64)
    l = r2.astype(BF16)
    return h, m, l


def kernel(q2_obs_scaled, amplitude_logits, volumes, filters, sigma,
           _trace=False, _tmpdir=None):
    from concourse.bass_utils import run_bass_kernel_spmd

    sig = float(np.asarray(sigma).reshape(()))
    minus_c = -0.5 / (sig + 0.001) ** 2
    c = -minus_c
    thr = np.sqrt(THR_LN / c)

    q = np.asarray(q2_obs_scaled, np.float64)                    # (B, P)
    lg = np.asarray(amplitude_logits, np.float64).reshape(B, F, P)
    vol = np.asarray(volumes, np.float64).reshape(V)
    fil = np.asarray(filters, np.float64).reshape(F)

    mx = lg.max(axis=2, keepdims=True)
    lnamp = lg - (mx + np.log(np.exp(lg - mx).sum(axis=2, keepdims=True)))

    # ---- schedule: windowed selection, global over the batch ----
    vperm = np.argsort(vol, kind="stable")
    vs = vol[vperm]
    xs = vs[:, None] * fil[None, :]                              # (V, F)
    sel = [None] * NCH                                           # (B, P) bool
    Ks_chunk = [0] * NCH
    for cid in range(NCH):
        f, vh = cid >> 1, cid & 1
        xw = xs[vh * 128:(vh + 1) * 128, f]
        lo, hi = xw.min() - thr, xw.max() + thr
        m = (q >= lo) & (q <= hi)                                # (B, P)
        sel[cid] = m
        # power-of-two K so same-size PSUM slots never straddle a bank
        n = int(m.sum(axis=1).max())
        Ks_chunk[cid] = next(k for k in (8, 16, 32, 64) if k >= n)
    order = sorted(range(NCH), key=lambda cix: -Ks_chunk[cix])
    sched = Schedule(Ks_chunk, order)
    nc = _get_nc(minus_c, sched)

    # ---- stationary x-side tile (shared by all cores) ----
    xst = np.zeros((108, sched.xtot), dtype=BF16)
    for cid in range(NCH):
        gi, band, slot = sched.place[cid]
        f, vh = cid >> 1, cid & 1
        xw = xs[vh * 128:(vh + 1) * 128, f]                      # (128,)
        x2h, x2m, x2l = _split3(xw * xw)
        xh, xm, xl = _split3(xw)
        ones = np.ones(128, dtype=BF16)
        rows = [x2h, x2m, x2l, xh, xh, xh, xm, xm, xl, ones, ones, ones]
        xo = sched.xoff[cid]
        for r, arr in enumerate(rows):
            xst[32 * band + r, xo:xo + 128] = arr

    # ---- per-core moving q-side tiles ----
    wh_a, wm_a, wl_a = _split3(-2.0 * q)                         # (B, P)
    phi = q[:, None, :] ** 2 - lnamp / c                         # (B, F, P)
    ph_a, pm_a, pl_a = _split3(phi)

    in_maps = []
    for i in range(NCORES):
        wmv = np.zeros((108, sched.wtot), dtype=BF16)
        # every pad/dummy column: exp -> 0
        for gi, (Kg, spb, cids) in enumerate(sched.groups):
            for idx, cid in enumerate(cids):
                band = idx % 4
                r0 = 32 * band
                key = cid if cid >= 0 else ("pad", gi, idx)
                wo = sched.woff[key]
                wmv[r0 + 0, wo:wo + 4 * Kg] = 1.0
                wmv[r0 + 1, wo:wo + 4 * Kg] = 1.0
                wmv[r0 + 2, wo:wo + 4 * Kg] = 1.0
                wmv[r0 + 9, wo:wo + 4 * Kg] = PAD_PHI
                if cid < 0:
                    continue
                f = cid >> 1
                for bl in range(B_LOC):
                    bg = B_LOC * i + bl
                    ps = np.nonzero(sel[cid][bg])[0]
                    n = len(ps)
                    col = wo + bl * Kg
                    wmv[r0 + 3, col:col + n] = wh_a[bg, ps]
                    wmv[r0 + 4, col:col + n] = wm_a[bg, ps]
                    wmv[r0 + 5, col:col + n] = wl_a[bg, ps]
                    wmv[r0 + 6, col:col + n] = wh_a[bg, ps]
                    wmv[r0 + 7, col:col + n] = wm_a[bg, ps]
                    wmv[r0 + 8, col:col + n] = wh_a[bg, ps]
                    wmv[r0 + 9, col:col + n] = ph_a[bg, f, ps]
                    wmv[r0 + 10, col:col + n] = pm_a[bg, f, ps]
                    wmv[r0 + 11, col:col + n] = pl_a[bg, f, ps]
        in_maps.append({"xst": xst, "wmv": wmv})

    kw = {}
    if _trace:
        kw = {"trace": True, "tmpdir": _tmpdir}
    res = run_bass_kernel_spmd(nc, in_maps, core_ids=list(range(NCORES)), **kw)

    # ---- host unpack: R[v'(sorted), rcol[cid]+b] -> out[b, v, f] ----
    vback = vperm.reshape(2, 128)                                # vh, v'
    out = np.empty((B, V, F), dtype=np.float32)
    rc = np.array([sched.rcol[cid] for cid in range(NCH)])       # (NCH,)
    for i in range(NCORES):
        R = res.results[i]["out"]                                # (128, rtot)
        cols = rc[:, None] + np.arange(B_LOC)[None, :]           # (NCH, 4)
        Rg = R[:, cols]                                          # (128,NCH,4)
        for bl in range(B_LOC):
            o = out[B_LOC * i + bl]                              # (V, F)
            g = Rg[:, :, bl].reshape(128, F, 2)                  # v', f, vh
            for vh in range(2):
                o[vback[vh], :] = g[:, :, vh]
    if _trace:
        return out, res
    return out


# revision 17
# speedup vs baseline: 1.7878x; 1.1321x over previous
"""Trainium2 Bass kernel for nn_ExtractionLayer (v4, 4-band windowed sparsity).

metric[b,v,f] = sum_p amp[b,f,p] * exp(-c*(vol[v]*filt[f] - q[b,p])^2)
  amp = softmax_p(logits[b,f,p]),  c = 0.5/(sigma+0.001)^2

Sharding: data-parallel over batch B=32 -> 4 b's per core on 8 cores.

Transposed layout: chunks (f, vh) put 128 v's (vol-sorted) on PSUM
partitions and selected (b,p) columns on the free axis. A K=12 bf16
matmul per chunk computes S = x^2 - 2qx + q^2 - lnamp/c (softmax amp
folded into the exponent via ln), ACT does E = exp(-c*S) in ~2048-col
group instructions, DVE does the segmented p-sum (one fp16 halving add
in 2x mode + one short reduce per group).

Windowed sparsity: exp(-c*d^2) < 1e-6 once |d| > sqrt(14/c), so each
chunk keeps only Ks = pow2ceil(max_b #{p: q[b,p] in x-window}) p-slots
per b (max over the GLOBAL batch so all 8 SPMD cores share one
schedule; pow2 Ks so equal PSUM slots never straddle a bank).

4 PE bands: chunks rotate tile_position row 0/32/64/96; band i writes
PSUM bank i of the group tile (concurrent row-tile matmuls must never
share a PSUM bank). The stationary/moving tiles hold band data at
partition blocks 0/32/64/96, which cuts per-partition DMA bytes ~2.5x
vs 2 bands -- input DMA then easily outruns the ~1.9us/group pipeline.

ALL small tensors are precomputed on host in fp64; the schedule is
baked per (sigma, selection counts) and cached.
"""

import sys

for _p in ("/opt/trn_rl_repo", "/root/.axon_site/_ro/trn_rl_repo"):
    if _p not in sys.path:
        sys.path.append(_p)

import numpy as np
import ml_dtypes

BF16 = ml_dtypes.bfloat16

B, V, F, P = 32, 256, 128, 64
NCORES = 8
B_LOC = B // NCORES          # 4 batches per core
NCH = 2 * F                  # 256 chunks: (f, vh)
NK = 12                      # matmul contraction rows
BANK = 512                   # psum cols per bank == per band-quarter
THR_LN = 14.0                # keep q with c*(x-q)^2 <= THR_LN at window edge
PAD_PHI = 100.0              # phi for padding columns -> exp(-c*100) == 0

_cache: dict = {}


class Schedule:
    """Data-dependent but core-independent processing plan.

    Chunks sorted by Ks desc, packed into groups of 4*spb slots
    (spb = 512 // (4*Kg) slots per bank, 4 banks). cid == -1 marks a
    dummy pad chunk (all-pad columns, output discarded).
    """

    def __init__(self, Ks_chunk, order):
        self.groups = []        # (Kg, spb, [cids (len 4*spb, -1 pads)])
        i = 0
        while i < NCH:
            Kg = Ks_chunk[order[i]]
            spb = BANK // (4 * Kg)
            if i == 0:
                spb = max(1, spb // 2)   # small first group: early EXP start
            cap = 4 * spb
            cids = order[i:i + cap]
            i += len(cids)
            cids = list(cids) + [-1] * (cap - len(cids))
            self.groups.append((Kg, spb, cids))
        # per-chunk placement
        self.place = {}         # cid -> (gi, band, slot)
        self.xoff = {}          # cid -> xst col offset (128 wide)
        self.woff = {}          # cid -> wmv col offset (4*Kg wide)
        self.rcol = {}          # cid -> R col base (4 wide)
        self.rbase = []
        xslot = 0
        wcur = 0
        racc = 0
        self.xslots = []        # per group: (xoff0, woff0) for DMA ranges
        for gi, (Kg, spb, cids) in enumerate(self.groups):
            self.rbase.append(racc)
            self.xslots.append((xslot * 128, wcur))
            for idx, cid in enumerate(cids):
                band, slot = idx % 4, idx // 4
                key = cid if cid >= 0 else ("pad", gi, idx)
                self.place[key] = (gi, band, slot)
                self.xoff[key] = (xslot + slot) * 128
                self.woff[key] = wcur + slot * 4 * Kg
                self.rcol[key] = racc + band * 4 * spb + slot * 4
            xslot += spb
            wcur += spb * 4 * Kg
            racc += 16 * spb
        self.xtot = xslot * 128
        self.wtot = wcur
        self.rtot = racc
        self.key = (tuple(Ks_chunk), tuple(order))


def _build(minus_c, sched):
    import concourse.tile as tile
    from concourse import bacc, mybir

    fp32 = mybir.dt.float32
    fp16 = mybir.dt.float16
    bf16 = mybir.dt.bfloat16
    AF = mybir.ActivationFunctionType
    OP = mybir.AluOpType
    import concourse.bass as bass

    nc = bacc.Bacc("TRN2", target_bir_lowering=False, debug=False,
                   num_devices=NCORES)

    d_xst = nc.dram_tensor("xst", [108, sched.xtot], bf16,
                           kind="ExternalInput")
    d_wmv = nc.dram_tensor("wmv", [108, sched.wtot], bf16,
                           kind="ExternalInput")
    d_out = nc.dram_tensor("out", [128, sched.rtot], fp32,
                           kind="ExternalOutput")

    ngroups = len(sched.groups)

    with tile.TileContext(nc) as tc:
        with (
            tc.tile_pool(name="const", bufs=1) as cp,
            tc.tile_pool(name="ering", bufs=2) as ep,
            tc.tile_pool(name="e2ring", bufs=2) as ep2,
            tc.tile_pool(name="e3ring", bufs=2) as ep3,
            tc.tile_pool(name="psS", bufs=2, space=bass.MemorySpace.PSUM) as psS,
        ):
            warm = cp.tile([1, 2], fp32, tag="warm")
            nc.vector.memset(warm[:, :], 0.0)
            nc.scalar.activation(warm[:, 0:1], warm[:, 1:2], AF.Exp)

            xst = cp.tile([108, sched.xtot], bf16, tag="xst")
            wmv = cp.tile([108, sched.wtot], bf16, tag="wmv")
            R = cp.tile([128, sched.rtot], fp32, tag="R")

            # input pieces by groups: fine first, then coarse
            gsz = [1, 1, 1, 1, 2, 2]
            while sum(gsz) < ngroups:
                gsz.append(min(3, ngroups - sum(gsz)))
            g0 = 0
            for ng in gsz:
                gb = min(g0 + ng, ngroups)
                x0, w0 = sched.xslots[g0]
                if gb < ngroups:
                    x1, w1 = sched.xslots[gb]
                else:
                    x1, w1 = sched.xtot, sched.wtot
                nc.sync.dma_start(xst[:, x0:x1], d_xst.ap()[:, x0:x1])
                nc.gpsimd.dma_start(wmv[:, w0:w1], d_wmv.ap()[:, w0:w1])
                g0 = gb

            ocursor = 0
            for gi in range(ngroups):
                Kg, spb, cids = sched.groups[gi]
                h = spb * 4 * Kg          # cols per bank (== 512 if full)
                sS = psS.tile([128, 4 * BANK], fp32, tag="S", name="sS")
                for idx, cid in enumerate(cids):
                    key = cid if cid >= 0 else ("pad", gi, idx)
                    band, slot = idx % 4, idx // 4
                    r0 = 32 * band
                    xo = sched.xoff[key]
                    wo = sched.woff[key]
                    pc = band * BANK + slot * 4 * Kg
                    nc.tensor.matmul(
                        sS[:, pc:pc + 4 * Kg],
                        xst[r0:r0 + NK, xo:xo + 128],
                        wmv[r0:r0 + NK, wo:wo + 4 * Kg],
                        start=True, stop=True,
                        tile_position=(r0, 0),
                    )
                E = ep.tile([128, 4 * BANK], fp16, tag="E", name="E")
                Sv = sS[:, :].rearrange("p (u x) -> p u x", u=4)[:, :, 0:h]
                Ev = E[:, :].rearrange("p (u x) -> p u x", u=4)[:, :, 0:h]
                nc.scalar.activation(Ev, Sv, AF.Exp, scale=float(minus_c))
                # p-sum: one fp16 halving add (2x mode) + one Kg/2 reduce
                nseg = spb * 4
                E4 = (E[:, :].rearrange("p (u y) -> p u y", u=4)
                      [:, :, 0:nseg * Kg]
                      .rearrange("p u (s x) -> p u s x", x=Kg))
                E2 = ep2.tile([128, 2 * BANK], fp16, tag="E2", name="E2")
                E2v = (E2[:, :].rearrange("p (u y) -> p u y", u=4)
                       [:, :, 0:nseg * (Kg // 2)]
                       .rearrange("p u (s x) -> p u s x", x=Kg // 2))
                nc.vector.tensor_tensor(E2v, E4[:, :, :, 0:Kg // 2],
                                        E4[:, :, :, Kg // 2:Kg], OP.add)
                red_in = E2v
                if Kg >= 32:
                    E3 = ep3.tile([128, BANK], fp16, tag="E3", name="E3")
                    E3v = (E3[:, :].rearrange("p (u y) -> p u y", u=4)
                           [:, :, 0:nseg * (Kg // 4)]
                           .rearrange("p u (s x) -> p u s x", x=Kg // 4))
                    nc.vector.tensor_tensor(E3v, E2v[:, :, :, 0:Kg // 4],
                                            E2v[:, :, :, Kg // 4:Kg // 2],
                                            OP.add)
                    red_in = E3v
                nc.vector.tensor_reduce(
                    R[:, sched.rbase[gi]:sched.rbase[gi] + 16 * spb]
                    .rearrange("p (u s) -> p u s", u=4),
                    red_in, mybir.AxisListType.X, OP.add)
                rend = sched.rbase[gi] + 16 * spb
                if gi % 3 == 2 or gi >= ngroups - 4:
                    nc.sync.dma_start(d_out.ap()[:, ocursor:rend],
                                      R[:, ocursor:rend])
                    ocursor = rend

    nc.compile()
    return nc


def _get_nc(minus_c, sched):
    key = (float(minus_c), sched.key)
    if key not in _cache:
        _cache[key] = _build(minus_c, sched)
    return _cache[key]


def _split3(v):
    """3-way bf16 split of an fp64 array: h + m + l ~= v to ~24 bits."""
    h = v.astype(BF16)
    r = v - h.astype(np.float64)
    m = r.astype(BF16)
    r2 = r - m.astype(np.float64)
    l = r2.astype(BF16)
    return h, m, l


def kernel(q2_obs_scaled, amplitude_logits, volumes, filters, sigma,
           _trace=False, _tmpdir=None):
    from concourse.bass_utils import run_bass_kernel_spmd

    sig = float(np.asarray(sigma).reshape(()))
    minus_c = -0.5 / (sig + 0.001) ** 2
    c = -minus_c
    thr = np.sqrt(THR_LN / c)

    q = np.asarray(q2_obs_scaled, np.float64)                    # (B, P)
    lg = np.asarray(amplitude_logits, np.float64).reshape(B, F, P)
    vol = np.asarray(volumes, np.float64).reshape(V)
    fil = np.asarray(filters, np.float64).reshape(F)

    mx = lg.max(axis=2, keepdims=True)
    lnamp = lg - (mx + np.log(np.exp(lg - mx).sum(axis=2, keepdims=True)))

    # ---- schedule: windowed selection, global over the batch ----
    vperm = np.argsort(vol, kind="stable")
    vs = vol[vperm]
    xs = vs[:, None] * fil[None, :]                              # (V, F)
    sel = [None] * NCH                                           # (B, P) bool
    Ks_chunk = [0] * NCH
    for cid in range(NCH):
        f, vh = cid >> 1, cid & 1
        xw = xs[vh * 128:(vh + 1) * 128, f]
        lo, hi = xw.min() - thr, xw.max() + thr
        m = (q >= lo) & (q <= hi)                                # (B, P)
        sel[cid] = m
        # multiple-of-8 K; uniform slots of 4K with spb=floor(512/4K)
        # slots per bank never straddle a PSUM bank
        n = int(m.sum(axis=1).max())
        Ks_chunk[cid] = max(8, -(-n // 8) * 8)
    order = sorted(range(NCH), key=lambda cix: -Ks_chunk[cix])
    sched = Schedule(Ks_chunk, order)
    nc = _get_nc(minus_c, sched)

    # ---- stationary x-side tile (shared by all cores) ----
    xst = np.zeros((108, sched.xtot), dtype=BF16)
    for cid in range(NCH):
        gi, band, slot = sched.place[cid]
        f, vh = cid >> 1, cid & 1
        xw = xs[vh * 128:(vh + 1) * 128, f]                      # (128,)
        x2h, x2m, x2l = _split3(xw * xw)
        xh, xm, xl = _split3(xw)
        ones = np.ones(128, dtype=BF16)
        rows = [x2h, x2m, x2l, xh, xh, xh, xm, xm, xl, ones, ones, ones]
        xo = sched.xoff[cid]
        for r, arr in enumerate(rows):
            xst[32 * band + r, xo:xo + 128] = arr

    # ---- per-core moving q-side tiles ----
    wh_a, wm_a, wl_a = _split3(-2.0 * q)                         # (B, P)
    phi = q[:, None, :] ** 2 - lnamp / c                         # (B, F, P)
    ph_a, pm_a, pl_a = _split3(phi)

    in_maps = []
    for i in range(NCORES):
        wmv = np.zeros((108, sched.wtot), dtype=BF16)
        # every pad/dummy column: exp -> 0
        for gi, (Kg, spb, cids) in enumerate(sched.groups):
            for idx, cid in enumerate(cids):
                band = idx % 4
                r0 = 32 * band
                key = cid if cid >= 0 else ("pad", gi, idx)
                wo = sched.woff[key]
                wmv[r0 + 0, wo:wo + 4 * Kg] = 1.0
                wmv[r0 + 1, wo:wo + 4 * Kg] = 1.0
                wmv[r0 + 2, wo:wo + 4 * Kg] = 1.0
                wmv[r0 + 9, wo:wo + 4 * Kg] = PAD_PHI
                if cid < 0:
                    continue
                f = cid >> 1
                for bl in range(B_LOC):
                    bg = B_LOC * i + bl
                    ps = np.nonzero(sel[cid][bg])[0]
                    n = len(ps)
                    col = wo + bl * Kg
                    wmv[r0 + 3, col:col + n] = wh_a[bg, ps]
                    wmv[r0 + 4, col:col + n] = wm_a[bg, ps]
                    wmv[r0 + 5, col:col + n] = wl_a[bg, ps]
                    wmv[r0 + 6, col:col + n] = wh_a[bg, ps]
                    wmv[r0 + 7, col:col + n] = wm_a[bg, ps]
                    wmv[r0 + 8, col:col + n] = wh_a[bg, ps]
                    wmv[r0 + 9, col:col + n] = ph_a[bg, f, ps]
                    wmv[r0 + 10, col:col + n] = pm_a[bg, f, ps]
                    wmv[r0 + 11, col:col + n] = pl_a[bg, f, ps]
        in_maps.append({"xst": xst, "wmv": wmv})

    kw = {}
    if _trace:
        kw = {"trace": True, "tmpdir": _tmpdir}
    res = run_bass_kernel_spmd(nc, in_maps, core_ids=list(range(NCORES)), **kw)

    # ---- host unpack: R[v'(sorted), rcol[cid]+b] -> out[b, v, f] ----
    vback = vperm.reshape(2, 128)                                # vh, v'
    out = np.empty((B, V, F), dtype=np.float32)
    rc = np.array([sched.rcol[cid] for cid in range(NCH)])       # (NCH,)
    for i in range(NCORES):
        R = res.results[i]["out"]                                # (128, rtot)
        cols = rc[:, None] + np.arange(B_LOC)[None, :]           # (NCH, 4)
        Rg = R[:, cols]                                          # (128,NCH,4)
        for bl in range(B_LOC):
            o = out[B_LOC * i + bl]                              # (V, F)
            g = Rg[:, :, bl].reshape(128, F, 2)                  # v', f, vh
            for vh in range(2):
                o[vback[vh], :] = g[:, :, vh]
    if _trace:
        return out, res
    return out


# revision 18
# speedup vs baseline: 1.8064x; 1.0104x over previous
"""Trainium2 Bass kernel for nn_ExtractionLayer (v4, 4-band windowed sparsity).

metric[b,v,f] = sum_p amp[b,f,p] * exp(-c*(vol[v]*filt[f] - q[b,p])^2)
  amp = softmax_p(logits[b,f,p]),  c = 0.5/(sigma+0.001)^2

Sharding: data-parallel over batch B=32 -> 4 b's per core on 8 cores.

Transposed layout: chunks (f, vh) put 128 v's (vol-sorted) on PSUM
partitions and selected (b,p) columns on the free axis. A K=12 bf16
matmul per chunk computes S = x^2 - 2qx + q^2 - lnamp/c (softmax amp
folded into the exponent via ln), ACT does E = exp(-c*S) in ~2048-col
group instructions, DVE does the segmented p-sum (one fp16 halving add
in 2x mode + one short reduce per group).

Windowed sparsity: exp(-c*d^2) < 1e-6 once |d| > sqrt(14/c), so each
chunk keeps only Ks = pow2ceil(max_b #{p: q[b,p] in x-window}) p-slots
per b (max over the GLOBAL batch so all 8 SPMD cores share one
schedule; pow2 Ks so equal PSUM slots never straddle a bank).

4 PE bands: chunks rotate tile_position row 0/32/64/96; band i writes
PSUM bank i of the group tile (concurrent row-tile matmuls must never
share a PSUM bank). The stationary/moving tiles hold band data at
partition blocks 0/32/64/96, which cuts per-partition DMA bytes ~2.5x
vs 2 bands -- input DMA then easily outruns the ~1.9us/group pipeline.

ALL small tensors are precomputed on host in fp64; the schedule is
baked per (sigma, selection counts) and cached.
"""

import sys

for _p in ("/opt/trn_rl_repo", "/root/.axon_site/_ro/trn_rl_repo"):
    if _p not in sys.path:
        sys.path.append(_p)

import numpy as np
import ml_dtypes

BF16 = ml_dtypes.bfloat16

B, V, F, P = 32, 256, 128, 64
NCORES = 8
B_LOC = B // NCORES          # 4 batches per core
NCH = 2 * F                  # 256 chunks: (f, vh)
NK = 12                      # matmul contraction rows
BANK = 512                   # psum cols per bank == per band-quarter
THR_LN = 14.0                # keep q with c*(x-q)^2 <= THR_LN at window edge
PAD_PHI = 100.0              # phi for padding columns -> exp(-c*100) == 0

_cache: dict = {}


class Schedule:
    """Data-dependent but core-independent processing plan.

    Chunks sorted by Ks desc, packed into groups of 4*spb slots
    (spb = 512 // (4*Kg) slots per bank, 4 banks). cid == -1 marks a
    dummy pad chunk (all-pad columns, output discarded).
    """

    def __init__(self, Ks_chunk, order):
        self.groups = []        # (Kg, spb, [cids (len 4*spb, -1 pads)])
        i = 0
        while i < NCH:
            Kg = Ks_chunk[order[i]]
            spb = BANK // (4 * Kg)
            if i == 0:
                spb = max(1, spb // 2)   # small first group: early EXP start
            cap = 4 * spb
            cids = order[i:i + cap]
            i += len(cids)
            cids = list(cids) + [-1] * (cap - len(cids))
            self.groups.append((Kg, spb, cids))
        # per-chunk placement
        self.place = {}         # cid -> (gi, band, slot)
        self.xoff = {}          # cid -> xst col offset (128 wide)
        self.woff = {}          # cid -> wmv col offset (4*Kg wide)
        self.rcol = {}          # cid -> R col base (4 wide)
        self.rbase = []
        xslot = 0
        wcur = 0
        racc = 0
        self.xslots = []        # per group: (xoff0, woff0) for DMA ranges
        for gi, (Kg, spb, cids) in enumerate(self.groups):
            self.rbase.append(racc)
            self.xslots.append((xslot * 128, wcur))
            for idx, cid in enumerate(cids):
                band, slot = idx % 4, idx // 4
                key = cid if cid >= 0 else ("pad", gi, idx)
                self.place[key] = (gi, band, slot)
                self.xoff[key] = (xslot + slot) * 128
                self.woff[key] = wcur + slot * 4 * Kg
                self.rcol[key] = racc + band * 4 * spb + slot * 4
            xslot += spb
            wcur += spb * 4 * Kg
            racc += 16 * spb
        self.xtot = xslot * 128
        self.wtot = wcur
        self.rtot = racc
        self.key = (tuple(Ks_chunk), tuple(order))


def _build(minus_c, sched):
    import concourse.tile as tile
    from concourse import bacc, mybir

    fp32 = mybir.dt.float32
    fp16 = mybir.dt.float16
    bf16 = mybir.dt.bfloat16
    AF = mybir.ActivationFunctionType
    OP = mybir.AluOpType
    import concourse.bass as bass

    nc = bacc.Bacc("TRN2", target_bir_lowering=False, debug=False,
                   num_devices=NCORES)

    d_xst = nc.dram_tensor("xst", [108, sched.xtot], bf16,
                           kind="ExternalInput")
    d_wmv = nc.dram_tensor("wmv", [108, sched.wtot], bf16,
                           kind="ExternalInput")
    d_out = nc.dram_tensor("out", [128, sched.rtot], fp32,
                           kind="ExternalOutput")

    ngroups = len(sched.groups)

    with tile.TileContext(nc) as tc:
        with (
            tc.tile_pool(name="const", bufs=1) as cp,
            tc.tile_pool(name="ering", bufs=2) as ep,
            tc.tile_pool(name="e2ring", bufs=2) as ep2,
            tc.tile_pool(name="e3ring", bufs=2) as ep3,
            tc.tile_pool(name="psS", bufs=2, space=bass.MemorySpace.PSUM) as psS,
        ):
            warm = cp.tile([1, 2], fp32, tag="warm")
            nc.vector.memset(warm[:, :], 0.0)
            zb = cp.tile([128, 1], fp32, tag="zb")
            nc.vector.memset(zb[:, :], 0.0)
            nc.scalar.activation(warm[:, 0:1], warm[:, 1:2], AF.Exp,
                                 bias=zb[0:1, 0:1])

            xst = cp.tile([108, sched.xtot], bf16, tag="xst")
            wmv = cp.tile([108, sched.wtot], bf16, tag="wmv")
            R = cp.tile([128, sched.rtot], fp32, tag="R")

            # input pieces by groups: fine first, then coarse
            gsz = [1, 1, 1, 1, 2, 2]
            while sum(gsz) < ngroups:
                gsz.append(min(3, ngroups - sum(gsz)))
            g0 = 0
            for ng in gsz:
                gb = min(g0 + ng, ngroups)
                x0, w0 = sched.xslots[g0]
                if gb < ngroups:
                    x1, w1 = sched.xslots[gb]
                else:
                    x1, w1 = sched.xtot, sched.wtot
                nc.sync.dma_start(xst[:, x0:x1], d_xst.ap()[:, x0:x1])
                nc.gpsimd.dma_start(wmv[:, w0:w1], d_wmv.ap()[:, w0:w1])
                g0 = gb

            ocursor = 0
            for gi in range(ngroups):
                Kg, spb, cids = sched.groups[gi]
                h = spb * 4 * Kg          # cols per bank (== 512 if full)
                sS = psS.tile([128, 4 * BANK], fp32, tag="S", name="sS")
                for idx, cid in enumerate(cids):
                    key = cid if cid >= 0 else ("pad", gi, idx)
                    band, slot = idx % 4, idx // 4
                    r0 = 32 * band
                    xo = sched.xoff[key]
                    wo = sched.woff[key]
                    pc = band * BANK + slot * 4 * Kg
                    nc.tensor.matmul(
                        sS[:, pc:pc + 4 * Kg],
                        xst[r0:r0 + NK, xo:xo + 128],
                        wmv[r0:r0 + NK, wo:wo + 4 * Kg],
                        start=True, stop=True,
                        tile_position=(r0, 0),
                    )
                E = ep.tile([128, 4 * BANK], fp16, tag="E", name="E")
                Sv = sS[:, :].rearrange("p (u x) -> p u x", u=4)[:, :, 0:h]
                Ev = E[:, :].rearrange("p (u x) -> p u x", u=4)[:, :, 0:h]
                nc.scalar.activation(Ev, Sv, AF.Exp, scale=float(minus_c),
                                     bias=zb[:, 0:1])
                # p-sum: one fp16 halving add (2x mode) + one Kg/2 reduce
                nseg = spb * 4
                E4 = (E[:, :].rearrange("p (u y) -> p u y", u=4)
                      [:, :, 0:nseg * Kg]
                      .rearrange("p u (s x) -> p u s x", x=Kg))
                E2 = ep2.tile([128, 2 * BANK], fp16, tag="E2", name="E2")
                E2v = (E2[:, :].rearrange("p (u y) -> p u y", u=4)
                       [:, :, 0:nseg * (Kg // 2)]
                       .rearrange("p u (s x) -> p u s x", x=Kg // 2))
                nc.vector.tensor_tensor(E2v, E4[:, :, :, 0:Kg // 2],
                                        E4[:, :, :, Kg // 2:Kg], OP.add)
                red_in = E2v
                if Kg >= 32:
                    E3 = ep3.tile([128, BANK], fp16, tag="E3", name="E3")
                    E3v = (E3[:, :].rearrange("p (u y) -> p u y", u=4)
                           [:, :, 0:nseg * (Kg // 4)]
                           .rearrange("p u (s x) -> p u s x", x=Kg // 4))
                    nc.vector.tensor_tensor(E3v, E2v[:, :, :, 0:Kg // 4],
                                            E2v[:, :, :, Kg // 4:Kg // 2],
                                            OP.add)
                    red_in = E3v
                nc.vector.tensor_reduce(
                    R[:, sched.rbase[gi]:sched.rbase[gi] + 16 * spb]
                    .rearrange("p (u s) -> p u s", u=4),
                    red_in, mybir.AxisListType.X, OP.add)
                rend = sched.rbase[gi] + 16 * spb
                if gi % 3 == 2 or gi >= ngroups - 4:
                    nc.sync.dma_start(d_out.ap()[:, ocursor:rend],
                                      R[:, ocursor:rend])
                    ocursor = rend

    nc.compile()
    return nc


def _get_nc(minus_c, sched):
    key = (float(minus_c), sched.key)
    if key not in _cache:
        _cache[key] = _build(minus_c, sched)
    return _cache[key]


def _split3(v):
    """3-way bf16 split of an fp64 array: h + m + l ~= v to ~24 bits."""
    h = v.astype(BF16)
    r = v - h.astype(np.float64)
    m = r.astype(BF16)
    r2 = r - m.astype(np.float64)
    l = r2.astype(BF16)
    return h, m, l


def kernel(q2_obs_scaled, amplitude_logits, volumes, filters, sigma,
           _trace=False, _tmpdir=None):
    from concourse.bass_utils import run_bass_kernel_spmd

    sig = float(np.asarray(sigma).reshape(()))
    minus_c = -0.5 / (sig + 0.001) ** 2
    c = -minus_c
    thr = np.sqrt(THR_LN / c)

    q = np.asarray(q2_obs_scaled, np.float64)                    # (B, P)
    lg = np.asarray(amplitude_logits, np.float64).reshape(B, F, P)
    vol = np.asarray(volumes, np.float64).reshape(V)
    fil = np.asarray(filters, np.float64).reshape(F)

    mx = lg.max(axis=2, keepdims=True)
    lnamp = lg - (mx + np.log(np.exp(lg - mx).sum(axis=2, keepdims=True)))

    # ---- schedule: windowed selection, global over the batch ----
    vperm = np.argsort(vol, kind="stable")
    vs = vol[vperm]
    xs = vs[:, None] * fil[None, :]                              # (V, F)
    sel = [None] * NCH                                           # (B, P) bool
    Ks_chunk = [0] * NCH
    for cid in range(NCH):
        f, vh = cid >> 1, cid & 1
        xw = xs[vh * 128:(vh + 1) * 128, f]
        lo, hi = xw.min() - thr, xw.max() + thr
        m = (q >= lo) & (q <= hi)                                # (B, P)
        sel[cid] = m
        # multiple-of-8 K; uniform slots of 4K with spb=floor(512/4K)
        # slots per bank never straddle a PSUM bank
        n = int(m.sum(axis=1).max())
        Ks_chunk[cid] = max(8, -(-n // 8) * 8)
    order = sorted(range(NCH), key=lambda cix: -Ks_chunk[cix])
    sched = Schedule(Ks_chunk, order)
    nc = _get_nc(minus_c, sched)

    # ---- stationary x-side tile (shared by all cores) ----
    xst = np.zeros((108, sched.xtot), dtype=BF16)
    for cid in range(NCH):
        gi, band, slot = sched.place[cid]
        f, vh = cid >> 1, cid & 1
        xw = xs[vh * 128:(vh + 1) * 128, f]                      # (128,)
        x2h, x2m, x2l = _split3(xw * xw)
        xh, xm, xl = _split3(xw)
        ones = np.ones(128, dtype=BF16)
        rows = [x2h, x2m, x2l, xh, xh, xh, xm, xm, xl, ones, ones, ones]
        xo = sched.xoff[cid]
        for r, arr in enumerate(rows):
            xst[32 * band + r, xo:xo + 128] = arr

    # ---- per-core moving q-side tiles ----
    wh_a, wm_a, wl_a = _split3(-2.0 * q)                         # (B, P)
    phi = q[:, None, :] ** 2 - lnamp / c                         # (B, F, P)
    ph_a, pm_a, pl_a = _split3(phi)

    in_maps = []
    for i in range(NCORES):
        wmv = np.zeros((108, sched.wtot), dtype=BF16)
        # every pad/dummy column: exp -> 0
        for gi, (Kg, spb, cids) in enumerate(sched.groups):
            for idx, cid in enumerate(cids):
                band = idx % 4
                r0 = 32 * band
                key = cid if cid >= 0 else ("pad", gi, idx)
                wo = sched.woff[key]
                wmv[r0 + 0, wo:wo + 4 * Kg] = 1.0
                wmv[r0 + 1, wo:wo + 4 * Kg] = 1.0
                wmv[r0 + 2, wo:wo + 4 * Kg] = 1.0
                wmv[r0 + 9, wo:wo + 4 * Kg] = PAD_PHI
                if cid < 0:
                    continue
                f = cid >> 1
                for bl in range(B_LOC):
                    bg = B_LOC * i + bl
                    ps = np.nonzero(sel[cid][bg])[0]
                    n = len(ps)
                    col = wo + bl * Kg
                    wmv[r0 + 3, col:col + n] = wh_a[bg, ps]
                    wmv[r0 + 4, col:col + n] = wm_a[bg, ps]
                    wmv[r0 + 5, col:col + n] = wl_a[bg, ps]
                    wmv[r0 + 6, col:col + n] = wh_a[bg, ps]
                    wmv[r0 + 7, col:col + n] = wm_a[bg, ps]
                    wmv[r0 + 8, col:col + n] = wh_a[bg, ps]
                    wmv[r0 + 9, col:col + n] = ph_a[bg, f, ps]
                    wmv[r0 + 10, col:col + n] = pm_a[bg, f, ps]
                    wmv[r0 + 11, col:col + n] = pl_a[bg, f, ps]
        in_maps.append({"xst": xst, "wmv": wmv})

    kw = {}
    if _trace:
        kw = {"trace": True, "tmpdir": _tmpdir}
    res = run_bass_kernel_spmd(nc, in_maps, core_ids=list(range(NCORES)), **kw)

    # ---- host unpack: R[v'(sorted), rcol[cid]+b] -> out[b, v, f] ----
    vback = vperm.reshape(2, 128)                                # vh, v'
    out = np.empty((B, V, F), dtype=np.float32)
    rc = np.array([sched.rcol[cid] for cid in range(NCH)])       # (NCH,)
    for i in range(NCORES):
        R = res.results[i]["out"]                                # (128, rtot)
        cols = rc[:, None] + np.arange(B_LOC)[None, :]           # (NCH, 4)
        Rg = R[:, cols]                                          # (128,NCH,4)
        for bl in range(B_LOC):
            o = out[B_LOC * i + bl]                              # (V, F)
            g = Rg[:, :, bl].reshape(128, F, 2)                  # v', f, vh
            for vh in range(2):
                o[vback[vh], :] = g[:, :, vh]
    if _trace:
        return out, res
    return out
